# revision 16
# baseline (speedup 1.0000x reference)
"""Trainium2 Bass kernel for nn_ContrastiveCenterLoss_M.

Math reduction
--------------
reference computes, per sample b and class c, a Mahalanobis distance between
the pooled-normalized hidden vector x_b (8-dim) and pooled-normalized class
center y_c (8-dim), where the 8x8 covariance is over the 200 points
{x_b (repeated 100x), y_0..y_99}:

    cov_b = A + beta d_b d_b^T,  A = S_y/199,  d_b = x_b - ybar,  beta = 50/199

A depends only on feature_center and is well-conditioned (cond ~1.9), so
pinv == inv and Sherman-Morrison collapses the per-sample pinv to a rank-1
correction of the shared M = inv(A).  All ybar terms are folded into host
constants so the device only needs x-dot-products (ones-row of the matmul
carries the per-class affine parts):

    ur_c  = u.(M y_c) = x.r_c - a_c        [matmul block 1, 100 cols]
    b2_c  = k2_c - 2 ur_c                  [matmul block 2, 100 cols]
    e0    = u.(M ybar) = x.c0 - q0         [matmul block 3, col 0]
    w_g   = (M u)_g   = x.(M e_g) - c0_g   [matmul block 3, cols 1:9]
    xws   = x.w = uw + e0 ;  uw = xws - e0
    gam   = 1/(1/beta + uw)
    m[b,c] = (b2_c + s2) - gam (ur_c - xws)^2 ,  s2 = xws + e0
    dis = sqrt(m); rowsum_b = sum_c dis; mt_b = m[b,y_b]
    host: loss_b = (C sqrt(mt) - rowsum)/(C-1); mean over B.

Perf notes (TimelineSim cost model):
 - matmuls/transpose in bf16 (1 cy/row vs 4 for fp32).
 - fused scalar_tensor_tensor+accum_out for sumsq / x.w / mask+reduce;
   one-hot mask + bf16 const cast on the idle Pool engine; PSUM evac of
   x^T on the idle DVE.  (pow/divide ALU ops fail the walrus ISA check;
   rsqrt stays as ACT-sqrt + DVE-reciprocal.)
 - first ACT op is a Sqrt so walrus loads the sqrt table set once, early,
   overlapped with the input DMA.
 - per-DMA fixed cost is ~2.2us (HWDGE 625 + DGE 650 + sem-prop 900), so
   exactly 3 input DMAs (h split 384/129 + const) and 1 output DMA.
NOTE: InstTensorTensorReduce and [p,1]-shaped DRAM outputs crash the exec
unit on this runtime -- avoided (TensorScalarPtr accum + [128,2] output).
Measured dead ends: m (or m+s2-fusion) in PSUM regresses ~0.5-0.7us
(DVE-write-to-PSUM/scheduling, despite ACT's cheaper PSUM read); bf16
d/q saves nothing in the cost model and multiplies rel-err 16x.
"""

import sys

if "/opt/trn_rl_repo" not in sys.path:
    sys.path.insert(0, "/opt/trn_rl_repo")

import numpy as np

B = 1024
D = 512
C = 100
POOL = 8
G = D // POOL          # 64
NCORES = 8
BS = B // NCORES       # 128 samples per core
BETA = (C / 2) / (2 * C - 1)   # 50/199
NCONST = 209           # [r|-a (100) | -2r|k2+2a (100) | c0|-q0 (1) | M|-c0 (8)]

_cache = {}


def _build():
    import concourse.mybir as mybir
    import concourse.tile as tile
    from concourse import bacc
    from concourse.masks import make_identity

    f32 = mybir.dt.float32
    bf16 = mybir.dt.bfloat16
    ALU = mybir.AluOpType
    ACT = mybir.ActivationFunctionType
    AX = mybir.AxisListType

    nc = bacc.Bacc(
        "TRN2",
        target_bir_lowering=False,
        debug=False,
        enable_asserts=False,
        num_devices=NCORES,
    )

    hidden_d = nc.dram_tensor("hidden_in", [BS, D + 1], f32, kind="ExternalInput")
    const_d = nc.dram_tensor("const_in", [POOL + 1, NCONST], f32, kind="ExternalInput")
    loss_d = nc.dram_tensor("loss_out", [BS, 2], f32, kind="ExternalOutput")

    with tile.TileContext(nc) as tc:
        with (
            tc.tile_pool(name="sb", bufs=1) as sb,
            tc.tile_pool(name="ps", bufs=1, space="PSUM") as ps,
        ):
            # ACT-table ordering hint: first ACT op is a Sqrt so walrus loads
            # the sqrt set (contains Square/Identity too) once, overlapped
            # with the DMA.
            warm = sb.tile([1, 1], f32)
            nc.vector.memset(warm[:, :], 1.0)
            nc.scalar.sqrt(out=warm[:, :], in_=warm[:, :])

            # hidden + labels (y packed as f32 col 512), asymmetric split: the
            # second (later-arriving) DMA is small so pooling finishes sooner
            SP1 = 6 * G
            h1 = sb.tile([BS, SP1], f32)
            h2 = sb.tile([BS, D - SP1 + 1], f32)
            nc.sync.dma_start(h1[:, :], hidden_d[:, 0:SP1])
            nc.sync.dma_start(h2[:, :], hidden_d[:, SP1:D + 1])
            ylab = h2[:, D - SP1:D - SP1 + 1]
            cst = sb.tile([POOL + 1, NCONST], f32)
            nc.sync.dma_start(cst[:, :], const_d[:, :])

            # constants with no deps: identity (PE transpose) + iota (one-hot)
            ident = sb.tile([BS, BS], bf16)
            make_identity(nc, ident[:, :])
            io_f = sb.tile([BS, C], f32)
            nc.gpsimd.iota(out=io_f[:, :], pattern=[[1, C]], base=0,
                           channel_multiplier=0, allow_small_or_imprecise_dtypes=True)

            # bf16 copy of the constants for the fast matmuls (Pool is idle)
            cstb = sb.tile([POOL + 1, NCONST], bf16)
            nc.gpsimd.tensor_copy(out=cstb[:, :], in_=cst[:, :])

            # ---- pool hidden into 8 groups of 64; x = pooled/||pooled|| ----
            # (the /G and the +1e-6 in the reference norm are negligible and
            # cancel / are dropped; see derivation in docstring)
            xn9 = sb.tile([BS, POOL + 1], bf16)   # [x | 1] for the PE ops
            nc.vector.memset(xn9[:, POOL:POOL + 1], 1.0)
            s8 = sb.tile([BS, POOL], f32)
            nc.vector.tensor_reduce(
                out=s8[:, 0:6],
                in_=h1[:, :].rearrange("p (g e) -> p g e", e=G),
                axis=AX.X, op=ALU.add,
            )
            nc.vector.tensor_reduce(
                out=s8[:, 6:POOL],
                in_=h2[:, 0:D - SP1].rearrange("p (g e) -> p g e", e=G),
                axis=AX.X, op=ALU.add,
            )
            sq = sb.tile([BS, POOL], f32)
            ss = sb.tile([BS, 1], f32)
            nc.vector.scalar_tensor_tensor(
                out=sq[:, :], in0=s8[:, :], scalar=1.0, in1=s8[:, :],
                op0=ALU.mult, op1=ALU.mult, accum_out=ss[:, :],
            )
            nv = sb.tile([BS, 1], f32)
            nc.scalar.sqrt(out=nv[:, :], in_=ss[:, :])
            rn = sb.tile([BS, 1], f32)
            nc.vector.reciprocal(out=rn[:, :], in_=nv[:, :])
            nc.vector.tensor_scalar(
                out=xn9[:, 0:POOL], in0=s8[:, :], scalar1=rn[:, 0:1],
                scalar2=None, op0=ALU.mult,
            )
            xnf = sb.tile([BS, POOL], f32)       # fp32 twin for the x.w dot
            nc.vector.tensor_scalar(
                out=xnf[:, :], in0=s8[:, :], scalar1=rn[:, 0:1],
                scalar2=None, op0=ALU.mult,
            )

            # one-hot mask of the true class: on gpsimd, off the DVE stream
            oh = sb.tile([BS, C], f32)
            nc.gpsimd.tensor_scalar(out=oh[:, :], in0=io_f[:, :], scalar1=ylab[:, 0:1],
                                    scalar2=None, op0=ALU.is_equal)

            # ---- x^T (+ones row) via PE transpose; evac PSUM->SBUF bf16
            # on DVE (idle here; ACT would start later) ----
            xnt_ps = ps.tile([POOL + 1, BS], bf16)
            nc.tensor.transpose(xnt_ps[:, :], xn9[:, :], ident[:, :])
            ut9 = sb.tile([POOL + 1, BS], bf16)
            nc.vector.tensor_copy(out=ut9[:, :], in_=xnt_ps[:, :])

            # ---- matmul blocks: small w/e0 first (feeds the per-sample
            # scalar chain), then b2, then ur last so d/q/m flow without
            # waiting on the b2 semaphore ----
            wps = ps.tile([BS, POOL + 1], f32)   # [e0 | w]
            nc.tensor.matmul(wps[:, :], ut9[:, :], cstb[:, 200:209])
            b2ps = ps.tile([BS, C], f32)         # k2_c - 2 ur_c
            nc.tensor.matmul(b2ps[:, :], ut9[:, :], cstb[:, C:2 * C])
            urps = ps.tile([BS, C], f32)         # ur_c
            nc.tensor.matmul(urps[:, :], ut9[:, :], cstb[:, 0:C])

            # ---- per-sample scalars (xws = x.w = uw + e0) ----
            xw = sb.tile([BS, POOL], f32)
            xws = sb.tile([BS, 1], f32)
            nc.vector.scalar_tensor_tensor(
                out=xw[:, :], in0=wps[:, 1:POOL + 1], scalar=1.0, in1=xnf[:, :],
                op0=ALU.mult, op1=ALU.mult, accum_out=xws[:, :],
            )
            e0 = wps[:, 0:1]
            den = sb.tile([BS, 1], f32)    # 1/beta + uw = (xws - e0) + 1/beta
            nc.vector.tensor_scalar(out=den[:, :], in0=xws[:, :], scalar1=e0,
                                    scalar2=1.0 / BETA, op0=ALU.subtract, op1=ALU.add)
            gam = sb.tile([BS, 1], f32)
            nc.vector.reciprocal(out=gam[:, :], in_=den[:, :])
            s2 = sb.tile([BS, 1], f32)     # uw + 2 e0 = xws + e0
            nc.vector.tensor_scalar(out=s2[:, :], in0=xws[:, :], scalar1=e0,
                                    scalar2=None, op0=ALU.add)

            # ---- m = (b2 + s2) - gam*(ur - xws)^2 ----
            d = sb.tile([BS, C], f32)
            nc.vector.tensor_scalar(out=d[:, :], in0=urps[:, :], scalar1=xws[:, 0:1],
                                    scalar2=None, op0=ALU.subtract)
            q = sb.tile([BS, C], f32)
            nc.vector.scalar_tensor_tensor(
                out=q[:, :], in0=d[:, :], scalar=gam[:, 0:1], in1=d[:, :],
                op0=ALU.mult, op1=ALU.mult,
            )
            m = sb.tile([BS, C], f32)
            nc.vector.scalar_tensor_tensor(
                out=m[:, :], in0=b2ps[:, :], scalar=s2[:, 0:1], in1=q[:, :],
                op0=ALU.add, op1=ALU.subtract,
            )

            # ---- outputs: rowsum of sqrt(m) on ACT; m[b,y] on DVE ----
            out2 = sb.tile([BS, 2], f32)
            dis = sb.tile([BS, C], f32)
            nc.scalar.activation(out=dis[:, :], in_=m[:, :], func=ACT.Sqrt,
                                 accum_out=out2[:, 0:1])
            mh = sb.tile([BS, C], f32)
            nc.vector.scalar_tensor_tensor(
                out=mh[:, :], in0=m[:, :], scalar=1.0, in1=oh[:, :],
                op0=ALU.mult, op1=ALU.mult, accum_out=out2[:, 1:2],
            )
            nc.sync.dma_start(loss_d[:, :], out2[:, :])

    nc.finalize()
    return nc


def _get_nc():
    if "nc" not in _cache:
        _cache["nc"] = _build()
    return _cache["nc"]


def _host_precompute(feature_center):
    fc = np.asarray(feature_center, dtype=np.float64)
    g = fc.reshape(C, POOL, G).mean(axis=2)                  # [100, 8]
    yn = g / (np.linalg.norm(g, axis=1, keepdims=True) + 1e-6)
    ybar = yn.mean(axis=0)
    z = yn - ybar
    A = (z.T @ z) / (2 * C - 1)
    M = np.linalg.inv(A)
    M = 0.5 * (M + M.T)
    r = yn @ M                                               # [100, 8]  M y_c
    a = r @ ybar                                             # ybar.M.y_c
    c0 = M @ ybar
    q0 = float(ybar @ c0)
    k2 = np.einsum('cd,ce,de->c', z, z, M)                   # z_c M z_c

    cp = np.zeros((POOL + 1, NCONST), dtype=np.float64)
    cp[0:POOL, 0:C] = r.T
    cp[POOL, 0:C] = -a
    cp[0:POOL, C:2 * C] = -2.0 * r.T
    cp[POOL, C:2 * C] = k2 + 2.0 * a
    cp[0:POOL, 2 * C] = c0
    cp[POOL, 2 * C] = -q0
    cp[0:POOL, 2 * C + 1:2 * C + 1 + POOL] = M
    cp[POOL, 2 * C + 1:2 * C + 1 + POOL] = -c0
    return cp.astype(np.float32)


def kernel(hidden, feature_center, y):
    from concourse import bass_utils

    ha = np.empty((B, D + 1), dtype=np.float32)
    ha[:, 0:D] = np.asarray(hidden, dtype=np.float32)
    ha[:, D] = np.asarray(y).astype(np.float32)
    cp = _host_precompute(feature_center)

    nc = _get_nc()
    in_maps = []
    for c in range(NCORES):
        in_maps.append({
            "hidden_in": ha[c * BS:(c + 1) * BS],
            "const_in": cp,
        })
    res = bass_utils.run_bass_kernel_spmd(nc, in_maps, core_ids=list(range(NCORES)))
    outs = np.concatenate([r["loss_out"] for r in res.results])  # [B, 2]
    rowsum = outs[:, 0].astype(np.float64)
    mt = outs[:, 1].astype(np.float64)
    loss = (C * np.sqrt(np.maximum(mt, 0.0)) - rowsum) / (C - 1)
    return np.float32(loss.mean())
